# revision 1
# baseline (speedup 1.0000x reference)
"""Trainium2 Bass kernel for HadamardTernaryLinear.

y = reshape( (FHT_g(x*alpha) @grouped w_q) -> FHT_h -> *beta ), with
w_q = BitNet-style absmean ternary quantization of weight.

Strategy: data-parallel over the 8192 tokens across 8 NeuronCores (1024
tokens/core, no collectives). Per core, a 5-pass TensorEngine pipeline in
bf16 (Hadamard and ternary weight matrices are exact +-1/0 in bf16; the
quant scale, alpha and beta are folded into f32 host prep / the final
f32 PSUM drain):

  P1 stat-FHTg : MM(lhsT=x-tile, rhs=I4(x)H32)   -> token-major  [tok, (i,h)]
  P2 T-regroup : MM(lhsT=sel,    rhs=I)          -> group-major  [i, tok] per h
  P3 stat-GM   : MM(lhsT=xb,     rhs=wqT[h])     -> token-major  [tok, (h,o)]
  P4 T-regroup : MM(lhsT=sel,    rhs=I)          -> A-layout     [(o',h), tok]
  P5 mov-FHTh  : MM(lhsT=I4(x)H32, rhs=ypa)      -> [(o',g), tok] f32 PSUM
  drain P5 on ScalarE with per-row scale = beta*quantscale/32, store f32.

Feature order on device is i-major (f' = i*32 + g), prepared host-side so
DMA transposes see contiguous 128-column blocks.
"""

import functools
import sys

for _p in ("/opt/trn_rl_repo",):
    if _p not in sys.path:
        sys.path.insert(0, _p)

import ml_dtypes
import numpy as np

import concourse.mybir as mybir
import concourse.tile as tile
from concourse import bacc
from concourse.bass_utils import run_bass_kernel_spmd

G = 32
IO = 128  # in_o
OO = 128  # out_o
D = G * IO  # 4096
NCORES = 8
B, T = 4, 2048
BT = B * T
TOKC = BT // NCORES  # tokens per core
ST = 512  # supertile tokens
NST = TOKC // ST

DTB = mybir.dt.bfloat16
DTF = mybir.dt.float32
BF16 = ml_dtypes.bfloat16


def _hadamard(n):
    H = np.array([[1.0]], dtype=np.float32)
    while H.shape[0] < n:
        H = np.block([[H, H], [H, -H]])
    return H  # +-1, symmetric


def build_body(nc, tc, xin, hm, idm, wqm, bc, yout, loop_r=1):
    """Emit the per-core program. All APs are DRAM tensors."""
    CH = ST // 128  # 128-token chunks per supertile

    with (
        tc.tile_pool(name="const", bufs=1) as cpool,
        tc.tile_pool(name="stage", bufs=1) as spool,
        tc.tile_pool(name="xa", bufs=1) as xapool,
        tc.tile_pool(name="yf", bufs=6) as ypool,
        tc.tile_pool(name="psum", bufs=6, space="PSUM") as pspool,
    ):
        hmt = cpool.tile([128, 128], DTB, tag="hm")
        nc.sync.dma_start(hmt[:], hm[:])
        idt = cpool.tile([128, 128], DTB, tag="id")
        nc.sync.dma_start(idt[:], idm[:])
        wqt = cpool.tile([128, G * OO], DTB, tag="wq")
        nc.sync.dma_start(wqt[:], wqm[:])
        bct = cpool.tile([128, G], DTF, tag="bc")
        nc.sync.dma_start(bct[:], bc[:])

        def supertile(st):
            t0 = st * ST
            # ---- loads: DMA-xbar transpose -> A-layout tiles [(i',g), tok]
            xa = []
            for k in range(32):
                xk = xapool.tile([128, ST], DTB, tag=f"xa{k}")
                nc.sync.dma_start_transpose(
                    xk[:], xin[t0 : t0 + ST, k * 128 : (k + 1) * 128]
                )
                xa.append(xk)

            # ---- P1: stationary FHT_g -> token-major tm_c [tok, i*32+h]
            tms = []
            for c in range(CH):
                tm = spool.tile([128, D], DTB, tag=f"tm{c}")
                for kq in range(8):
                    ps = pspool.tile([128, 512], DTF, tag="ps")
                    for kk in range(4):
                        k = kq * 4 + kk
                        nc.tensor.matmul(
                            ps[:, kk * 128 : (kk + 1) * 128],
                            lhsT=xa[k][:, c * 128 : (c + 1) * 128],
                            rhs=hmt[:],
                            start=True,
                            stop=True,
                        )
                    nc.vector.tensor_copy(tm[:, kq * 512 : (kq + 1) * 512], ps[:])
                tms.append(tm)

            # ---- P2: transpose-regroup -> xb_h [i, tok] (group-major)
            xb = []
            for h in range(32):
                ps = pspool.tile([128, ST], DTF, tag="ps")
                for c in range(CH):
                    sel = tms[c].rearrange("p (i h) -> p h i", h=32)[:, h, :]
                    nc.tensor.matmul(
                        ps[:, c * 128 : (c + 1) * 128],
                        lhsT=sel,
                        rhs=idt[:],
                        start=True,
                        stop=True,
                    )
                xbh = spool.tile([128, ST], DTB, tag=f"xb{h}")
                nc.scalar.copy(xbh[:], ps[:])
                xb.append(xbh)

            # ---- P3: stationary grouped matmul -> token-major tm2_c.
            # Drain scatters to o-major column order (col = o*32 + h) so P4's
            # weight selection is a contiguous 128-column slice (walrus allows
            # only 2D APs on the matmul stationary operand).
            tm2s = []
            for c in range(CH):
                tm2 = spool.tile([128, D], DTB, tag=f"tm2_{c}")
                tm2v = tm2.rearrange("p (o h) -> p h o", h=32)
                for hq in range(8):
                    ps = pspool.tile([128, 512], DTF, tag="ps")
                    for hh in range(4):
                        h = hq * 4 + hh
                        nc.tensor.matmul(
                            ps[:, hh * 128 : (hh + 1) * 128],
                            lhsT=xb[h][:, c * 128 : (c + 1) * 128],
                            rhs=wqt[:, h * 128 : (h + 1) * 128],
                            start=True,
                            stop=True,
                        )
                    nc.vector.tensor_copy(tm2v[:, hq * 4 : (hq + 1) * 4, :], ps[:])
                tm2s.append(tm2)

            # ---- P4: transpose-regroup -> ypa_m [(o',h), tok]
            ypa = []
            for m in range(32):
                ps = pspool.tile([128, ST], DTF, tag="ps")
                for c in range(CH):
                    nc.tensor.matmul(
                        ps[:, c * 128 : (c + 1) * 128],
                        lhsT=tm2s[c][:, m * 128 : (m + 1) * 128],
                        rhs=idt[:],
                        start=True,
                        stop=True,
                    )
                ym = spool.tile([128, ST], DTB, tag=f"ypa{m}")
                if m % 2 == 0:
                    nc.vector.tensor_copy(ym[:], ps[:])
                else:
                    nc.scalar.copy(ym[:], ps[:])
                ypa.append(ym)

            # ---- P5: moving FHT_h -> [(o',g), tok] f32; drain with beta scale
            for m in range(32):
                ps = pspool.tile([128, ST], DTF, tag="ps")
                nc.tensor.matmul(ps[:], lhsT=hmt[:], rhs=ypa[m][:], start=True, stop=True)
                yf = ypool.tile([128, ST], DTF, tag="yf")
                nc.scalar.activation(
                    yf[:],
                    ps[:],
                    mybir.ActivationFunctionType.Copy,
                    scale=bct[:, m : m + 1],
                )
                nc.sync.dma_start(yout[m * 128 : (m + 1) * 128, t0 : t0 + ST], yf[:])

        if loop_r == 1:
            for st in range(NST):
                supertile(st)
        else:
            with tc.For_i(0, loop_r, 1):
                for st in range(NST):
                    supertile(st)


@functools.lru_cache(maxsize=4)
def build_program(loop_r=1):
    nc = bacc.Bacc("TRN2", target_bir_lowering=False, debug=False)
    xin = nc.dram_tensor("xin", [TOKC, D], DTB, kind="ExternalInput").ap()
    hm = nc.dram_tensor("hmat", [128, 128], DTB, kind="ExternalInput").ap()
    idm = nc.dram_tensor("ident", [128, 128], DTB, kind="ExternalInput").ap()
    wqm = nc.dram_tensor("wqm", [128, G * OO], DTB, kind="ExternalInput").ap()
    bc = nc.dram_tensor("betacol", [128, G], DTF, kind="ExternalInput").ap()
    yout = nc.dram_tensor("yout", [D, TOKC], DTF, kind="ExternalOutput").ap()
    with tile.TileContext(nc) as tc:
        build_body(nc, tc, xin, hm, idm, wqm, bc, yout, loop_r=loop_r)
    nc.compile()
    return nc


def host_prep(x, weight, alpha, beta):
    """Returns (in_maps, decode_info). Pure f32 numpy glue + bf16 casts."""
    H = _hadamard(G)  # [g,h] +-1

    w = np.asarray(weight, dtype=np.float32)
    scale = np.float32(np.mean(np.abs(w))) + np.float32(1e-8)
    wq3 = np.clip(np.round(w / scale), -1.0, 1.0).astype(np.float32)  # [h,o,i] in {-1,0,1}

    # x * alpha, reorder features to i-major (f' = i*32+g)
    xp = np.asarray(x, dtype=np.float32).reshape(BT, G, IO) * np.asarray(
        alpha, dtype=np.float32
    )[None]
    xp = np.ascontiguousarray(xp.transpose(0, 2, 1)).reshape(BT, D)
    xin_all = xp.astype(BF16)

    hmat = np.kron(np.eye(4, dtype=np.float32), H).astype(BF16)  # [(i',g),(i'',h)]
    ident = np.eye(128, dtype=np.float32).astype(BF16)
    wq_sb = np.ascontiguousarray(wq3.transpose(2, 0, 1)).reshape(IO, G * OO).astype(BF16)  # [i,(h,o)]

    beta_f = np.asarray(beta, dtype=np.float32) * (scale / np.float32(G))  # [g,o]
    # betacol[p = o'*32+g, m] = beta_f[g, 4m+o']
    bc = np.ascontiguousarray(
        beta_f.T.reshape(G, 4, G).transpose(1, 2, 0)
    ).reshape(128, G).astype(np.float32)

    in_maps = []
    for c in range(NCORES):
        in_maps.append(
            {
                "xin": xin_all[c * TOKC : (c + 1) * TOKC],
                "hmat": hmat,
                "ident": ident,
                "wqm": wq_sb,
                "betacol": bc,
            }
        )
    return in_maps


def host_post(results):
    ydev = np.stack([r["yout"] for r in results])  # [8, 4096, 1024] f32
    # row r = m*128 + o'*32 + g  ->  feature (g, o = 4m+o'); want y[tok, g*128+o]
    y = ydev.reshape(NCORES, G, 4, G, TOKC)  # [c, m, o', g, tok]
    y = y.transpose(0, 4, 3, 1, 2)  # [c, tok, g, m, o']
    y = np.ascontiguousarray(y).reshape(BT, D)
    return y.reshape(B, T, D)


def kernel(x, weight, alpha, beta):
    nc = build_program(loop_r=1)
    in_maps = host_prep(x, weight, alpha, beta)
    res = run_bass_kernel_spmd(nc, in_maps, core_ids=list(range(NCORES)))
    return host_post(res.results)



# revision 14
# speedup vs baseline: 51.1018x; 51.1018x over previous
"""Trainium2 Bass kernel for HadamardTernaryLinear.

y = reshape( (FHT_g(x*alpha) @grouped w_q) -> FHT_h -> *beta ), with
w_q = BitNet-style absmean ternary quantization of weight.

Strategy: data-parallel over the 8192 tokens across 8 NeuronCores (1024
tokens/core, no collectives). Host pre-transposes x to feature-major
[D, tok] with i-major feature order (f' = i*32 + g), so all device DMA is
plain contiguous. Per core, a 5-pass TensorEngine pipeline in bf16
(Hadamard and ternary weights are exact +-1/0 in bf16; quant scale and
beta are folded into the final FHT stationary):

  P1 FHT_g   : MM(lhsT=xa-slice, rhs=I4(x)H32)  -> token-major [tok,(i,h)]
  P2 T-regrp : PE-transpose(sel cols h)         -> g-major [i, tok] per h (bf16 PSUM)
  P3 GMM     : MM(lhsT=xb-slice, rhs=wqT[h])    -> token-major, o-major col scatter
  P4 T-regrp : PE-transpose(contig cols m)      -> A-layout [(o',h), tok] (bf16 PSUM)
  P5 FHT_h   : MM(lhsT=I4(x)H32 * beta_scale)   -> [(o',g), tok] f32 PSUM

The five passes are software-pipelined across supertiles (stage s of
supertile t emits in slot t+s) so the TensorEngine stays dense while
PSUM drains trail on DVE / Act (GPSIMD cannot read PSUM). Drains are
dispatched to whichever of DVE/Act has less accumulated estimated time;
all-bf16 drains are cheaper on DVE (2x 16-bit mode). bf16 output, one
merged output DMA per supertile on the Pool queue.
"""

import functools
import sys

for _p in ("/opt/trn_rl_repo",):
    if _p not in sys.path:
        sys.path.insert(0, _p)

import ml_dtypes
import numpy as np

import concourse.mybir as mybir
import concourse.tile as tile
from concourse import bacc
from concourse.bass_utils import run_bass_kernel_spmd

G = 32
IO = 128  # in_o
OO = 128  # out_o
D = G * IO  # 4096
NCORES = 8
B, T = 4, 2048
BT = B * T
TOKC = BT // NCORES  # tokens per core
ST = 256  # supertile tokens
NST = TOKC // ST
CH = ST // 128  # 128-token chunks per supertile

DTB = mybir.dt.bfloat16
DTF = mybir.dt.float32
BF16 = ml_dtypes.bfloat16


def _hadamard(n):
    H = np.array([[1.0]], dtype=np.float32)
    while H.shape[0] < n:
        H = np.block([[H, H], [H, -H]])
    return H  # +-1, symmetric


class _Drain:
    """Cost-balancing drain dispatcher over DVE / Act.

    GPSIMD cannot read PSUM. DVE gets 2x on all-16-bit packed copies, so
    bf16 tiles are cheaper there; otherwise pick the engine with less
    accumulated estimated time.
    """

    def __init__(self, nc):
        self.nc = nc
        self.t = [0.0, 0.0]  # DVE, Act accumulated ns

    def __call__(self, out, in_):
        cols = in_.free_size()
        if in_.dtype == DTB:
            dve, act = cols / 2 / 0.96 + 130, cols / 1.2 + 185
        else:
            dve, act = cols / 0.96 + 130, cols / 1.2 + 185
        if self.t[0] + dve <= self.t[1] + act:
            self.t[0] += dve
            self.nc.vector.tensor_copy(out, in_)
        else:
            self.t[1] += act
            self.nc.scalar.copy(out, in_)


def build_body(nc, tc, xin, hm, idm, wqm, hbm, yout, loop_r=1):
    with (
        tc.tile_pool(name="const", bufs=1) as cpool,
        tc.tile_pool(name="xa", bufs=1) as xapool,
        tc.tile_pool(name="tmp", bufs=1) as tpool,
        tc.tile_pool(name="mid", bufs=1) as mpool,
        tc.tile_pool(name="yf", bufs=1) as ypool,
        tc.tile_pool(name="psum", bufs=1, space="PSUM") as pspool,
    ):
        hmt = cpool.tile([128, 128], DTB, tag="hm")
        nc.sync.dma_start(hmt[:], hm[:])
        idt = cpool.tile([128, 128], DTB, tag="id")
        nc.sync.dma_start(idt[:], idm[:])
        wqt = cpool.tile([128, G * OO], DTB, tag="wq")
        nc.sync.dma_start(wqt[:], wqm[:])
        hbt = cpool.tile([128, G * OO], DTB, tag="hb")
        nc.sync.dma_start(hbt[:], hbm[:])

        def body():
            rr = _Drain(nc)
            # contiguous loads: xa_oct[q][p, kk*TOKC + t] = xin[(8q+kk)*128+p, t].
            # One tile + one DMA per octet of k, spread across four DGE queues
            # so the head latency is ~1/4 and iteration n+1 prefetches during n.
            xin_v = xin.rearrange("(k p) t -> p k t", p=128)
            dma_engs = [nc.sync, nc.scalar, nc.gpsimd, nc.sync]
            xa_oct = []
            for q in range(4):
                xo = xapool.tile([128, 8 * TOKC], DTB, tag=f"xa{q}", name=f"xa{q}")
                xo_v = xo.rearrange("p (k t) -> p k t", k=8)
                dma_engs[q].dma_start(xo_v[:], xin_v[:, q * 8 : (q + 1) * 8, :])
                xa_oct.append(xo)

            def xas(k, lo, hi):
                return xa_oct[k // 8][:, (k % 8) * TOKC + lo : (k % 8) * TOKC + hi]

            state = [dict() for _ in range(NST)]

            def p1(st):
                t0 = st * ST
                s = state[st]
                s["tm"] = []
                for c in range(CH):
                    tm = tpool.tile([128, D], DTB, tag=f"tm{c}", name=f"tm{c}")
                    for q in range(8):
                        ps = pspool.tile([128, 512], DTF, tag="psA", name="psA", bufs=3)
                        for kk in range(4):
                            k = q * 4 + kk
                            nc.tensor.matmul(
                                ps[:, kk * 128 : (kk + 1) * 128],
                                lhsT=xas(k, t0 + c * 128, t0 + (c + 1) * 128),
                                rhs=hmt[:],
                                start=True,
                                stop=True,
                            )
                        rr(tm[:, q * 512 : (q + 1) * 512], ps[:])
                    s["tm"].append(tm)

            def p2(st):
                s = state[st]
                xb_all = mpool.tile([128, 32 * ST], DTB, tag="xb", name="xb_all")
                for hg in range(16):  # 2 h per psum tile
                    ps = pspool.tile([128, 512], DTB, tag="psB", name="psB", bufs=3)
                    for j in range(2):
                        h = 2 * hg + j
                        for c in range(CH):
                            sel = s["tm"][c].rearrange("p (i h) -> p h i", h=32)[:, h, :]
                            nc.tensor.transpose(
                                ps[:, j * ST + c * 128 : j * ST + (c + 1) * 128],
                                sel,
                                idt[:],
                            )
                    rr(xb_all[:, hg * 512 : (hg + 1) * 512], ps[:])
                s["xb"] = xb_all

            def p3(st):
                s = state[st]
                xb_all = s["xb"]
                s["tm2"] = []
                for c in range(CH):
                    tm2 = tpool.tile([128, D], DTB, tag=f"tm2_{c}", name=f"tm2_{c}")
                    tm2v = tm2.rearrange("p (o h) -> p h o", h=32)
                    for hq in range(8):  # 4 h per psum tile
                        ps = pspool.tile([128, 512], DTF, tag="psA", name="psA", bufs=3)
                        for hh in range(4):
                            h = hq * 4 + hh
                            nc.tensor.matmul(
                                ps[:, hh * 128 : (hh + 1) * 128],
                                lhsT=xb_all[
                                    :, h * ST + c * 128 : h * ST + (c + 1) * 128
                                ],
                                rhs=wqt[:, h * 128 : (h + 1) * 128],
                                start=True,
                                stop=True,
                            )
                        rr(tm2v[:, hq * 4 : (hq + 1) * 4, :], ps[:])
                    s["tm2"].append(tm2)

            def p4(st):
                s = state[st]
                ypa_all = mpool.tile([128, 32 * ST], DTB, tag="ypa", name="ypa_all")
                for mg in range(16):
                    ps = pspool.tile([128, 512], DTB, tag="psB", name="psB", bufs=3)
                    for j in range(2):
                        m = 2 * mg + j
                        for c in range(CH):
                            nc.tensor.transpose(
                                ps[:, j * ST + c * 128 : j * ST + (c + 1) * 128],
                                s["tm2"][c][:, m * 128 : (m + 1) * 128],
                                idt[:],
                            )
                    rr(ypa_all[:, mg * 512 : (mg + 1) * 512], ps[:])
                s["ypa"] = ypa_all

            def p5(st):
                t0 = st * ST
                s = state[st]
                ypa_all = s["ypa"]
                yf_all = ypool.tile([128, 32 * ST], DTB, tag="yf", name="yf_all")
                for mp in range(16):
                    ps = pspool.tile([128, 512], DTF, tag="psC", name="psC", bufs=2)
                    for j in range(2):
                        m = 2 * mp + j
                        nc.tensor.matmul(
                            ps[:, j * ST : (j + 1) * ST],
                            lhsT=hbt[:, m * 128 : (m + 1) * 128],
                            rhs=ypa_all[:, m * ST : (m + 1) * ST],
                            start=True,
                            stop=True,
                        )
                    rr(yf_all[:, mp * 512 : (mp + 1) * 512], ps[:])
                yout_v = yout.rearrange("(m p) t -> p m t", p=128)
                yf_v = yf_all.rearrange("p (m t) -> p m t", m=32)
                nc.gpsimd.dma_start(yout_v[:, :, t0 : t0 + ST], yf_v[:])

            stages = [p1, p2, p3, p4, p5]
            # software pipeline: slot t runs stage s of supertile t-s,
            # oldest stage first
            for t in range(NST + 4):
                for s in range(4, -1, -1):
                    st = t - s
                    if 0 <= st < NST:
                        stages[s](st)

        if loop_r == 1:
            body()
        else:
            with tc.For_i(0, loop_r, 1):
                body()


@functools.lru_cache(maxsize=4)
def build_program(loop_r=1):
    nc = bacc.Bacc("TRN2", target_bir_lowering=False, debug=False)
    xin = nc.dram_tensor("xin", [D, TOKC], DTB, kind="ExternalInput").ap()
    hm = nc.dram_tensor("hmat", [128, 128], DTB, kind="ExternalInput").ap()
    idm = nc.dram_tensor("ident", [128, 128], DTB, kind="ExternalInput").ap()
    wqm = nc.dram_tensor("wqm", [128, G * OO], DTB, kind="ExternalInput").ap()
    hbm = nc.dram_tensor("hbm", [128, G * OO], DTB, kind="ExternalInput").ap()
    yout = nc.dram_tensor("yout", [D, TOKC], DTB, kind="ExternalOutput").ap()
    with tile.TileContext(nc) as tc:
        build_body(nc, tc, xin, hm, idm, wqm, hbm, yout, loop_r=loop_r)
    nc.compile()
    return nc


def host_prep(x, weight, alpha, beta):
    """Pure f32 numpy glue + bf16 casts. Returns per-core input maps."""
    H = _hadamard(G)  # [g,h] +-1

    w = np.asarray(weight, dtype=np.float32)
    scale = np.float32(np.mean(np.abs(w))) + np.float32(1e-8)
    wq3 = np.clip(np.round(w / scale), -1.0, 1.0).astype(np.float32)  # [h,o,i]

    # x * alpha, reorder features to i-major (f' = i*32+g), transpose to [D, BT]
    xp = np.asarray(x, dtype=np.float32).reshape(BT, G, IO) * np.asarray(
        alpha, dtype=np.float32
    )[None]
    xt = np.ascontiguousarray(xp.transpose(2, 1, 0)).reshape(D, BT)  # rows i*32+g
    xt = xt.astype(BF16)

    hmat = np.kron(np.eye(4, dtype=np.float32), H).astype(BF16)  # [(i',g),(i'',h)]
    ident = np.eye(128, dtype=np.float32).astype(BF16)
    wq_sb = np.ascontiguousarray(wq3.transpose(2, 0, 1)).reshape(IO, G * OO).astype(BF16)

    # hbm[p=(o''*32+h), m*128 + o'*32 + g] = delta(o'',o') * H[h,g] * bs[4m+o', g]
    bs = np.asarray(beta, dtype=np.float32).T * (scale / np.float32(G))  # [o, g]
    hb = np.zeros((128, G * OO), dtype=np.float32)
    base = np.kron(np.eye(4, dtype=np.float32), H)  # [(o'',h), (o',g)]
    # columns of block m: j = o'*32 + g, scale = bs[4m+o', g]
    for m in range(32):
        colscale = bs[4 * m : 4 * m + 4, :].reshape(128)
        hb[:, m * 128 : (m + 1) * 128] = base * colscale[None, :]
    hb = hb.astype(BF16)

    in_maps = []
    for c in range(NCORES):
        in_maps.append(
            {
                "xin": np.ascontiguousarray(xt[:, c * TOKC : (c + 1) * TOKC]),
                "hmat": hmat,
                "ident": ident,
                "wqm": wq_sb,
                "hbm": hb,
            }
        )
    return in_maps


def host_post(results):
    ydev = np.stack([np.asarray(r["yout"], dtype=np.float32) for r in results])
    # row r = m*128 + o'*32 + g  ->  feature (g, o = 4m+o'); want y[tok, g*128+o]
    y = ydev.reshape(NCORES, G, 4, G, TOKC)  # [c, m, o', g, tok]
    y = y.transpose(0, 4, 3, 1, 2)  # [c, tok, g, m, o']
    y = np.ascontiguousarray(y).reshape(BT, D)
    return y.reshape(B, T, D)


def kernel(x, weight, alpha, beta):
    nc = build_program(loop_r=1)
    in_maps = host_prep(x, weight, alpha, beta)
    res = run_bass_kernel_spmd(nc, in_maps, core_ids=list(range(NCORES)))
    return host_post(res.results)


# revision 16
# speedup vs baseline: 57.7478x; 1.1301x over previous
"""Trainium2 Bass kernel for HadamardTernaryLinear.

y = reshape( (FHT_g(x*alpha) @grouped w_q) -> FHT_h -> *beta ), with
w_q = BitNet-style absmean ternary quantization of weight.

Strategy: data-parallel over the 8192 tokens across 8 NeuronCores (1024
tokens/core, no collectives). Host pre-transposes x to feature-major
[D, tok] with i-major feature order (f' = i*32 + g), so all device DMA is
plain contiguous. Per core, a 5-pass TensorEngine pipeline in bf16
(Hadamard and ternary weights are exact +-1/0 in bf16; quant scale and
beta are folded into the final FHT stationary):

  P1 FHT_g   : MM(lhsT=xa-slice, rhs=I4(x)H32)  -> token-major [tok,(i,h)]
  P2 T-regrp : PE-transpose(sel cols h)         -> g-major [i, tok] per h (bf16 PSUM)
  P3 GMM     : MM(lhsT=xb-slice, rhs=wqT[h])    -> token-major, o-major col scatter
  P4 T-regrp : PE-transpose(contig cols m)      -> A-layout [(o',h), tok] (bf16 PSUM)
  P5 FHT_h   : MM(lhsT=I4(x)H32 * beta_scale)   -> [(o',g), tok] f32 PSUM

The five passes are software-pipelined across supertiles (stage s of
supertile t emits in slot t+s) so the TensorEngine stays dense while
PSUM drains trail on DVE / Act (GPSIMD cannot read PSUM). Drains are
dispatched to whichever of DVE/Act has less accumulated estimated time;
all-bf16 drains are cheaper on DVE (2x 16-bit mode). bf16 output, one
merged output DMA per supertile on the Pool queue.
"""

import functools
import sys

for _p in ("/opt/trn_rl_repo",):
    if _p not in sys.path:
        sys.path.insert(0, _p)

import ml_dtypes
import numpy as np

import concourse.mybir as mybir
import concourse.tile as tile
from concourse import bacc
from concourse.bass_utils import run_bass_kernel_spmd

G = 32
IO = 128  # in_o
OO = 128  # out_o
D = G * IO  # 4096
NCORES = 8
B, T = 4, 2048
BT = B * T
TOKC = BT // NCORES  # tokens per core
ST = 128  # supertile tokens
NST = TOKC // ST
CH = ST // 128  # 128-token chunks per supertile

DTB = mybir.dt.bfloat16
DTF = mybir.dt.float32
BF16 = ml_dtypes.bfloat16


def _hadamard(n):
    H = np.array([[1.0]], dtype=np.float32)
    while H.shape[0] < n:
        H = np.block([[H, H], [H, -H]])
    return H  # +-1, symmetric


class _Drain:
    """Cost-balancing drain dispatcher over DVE / Act.

    GPSIMD cannot read PSUM. DVE gets 2x on all-16-bit packed copies, so
    bf16 tiles are cheaper there; otherwise pick the engine with less
    accumulated estimated time.
    """

    def __init__(self, nc):
        self.nc = nc
        self.t = [0.0, 0.0]  # DVE, Act accumulated ns

    def __call__(self, out, in_):
        cols = in_.free_size()
        if in_.dtype == DTB:
            dve, act = cols / 2 / 0.96 + 130, cols / 1.2 + 185
        else:
            dve, act = cols / 0.96 + 130, cols / 1.2 + 185
        if self.t[0] + dve <= self.t[1] + act:
            self.t[0] += dve
            self.nc.vector.tensor_copy(out, in_)
        else:
            self.t[1] += act
            self.nc.scalar.copy(out, in_)


def build_body(nc, tc, xin, hm, idm, wqm, hbm, yout, loop_r=1):
    with (
        tc.tile_pool(name="const", bufs=1) as cpool,
        tc.tile_pool(name="xa", bufs=1) as xapool,
        tc.tile_pool(name="tmp", bufs=2) as tpool,
        tc.tile_pool(name="mid", bufs=2) as mpool,
        tc.tile_pool(name="yf", bufs=2) as ypool,
        tc.tile_pool(name="psum", bufs=1, space="PSUM") as pspool,
    ):
        hmt = cpool.tile([128, 128], DTB, tag="hm")
        nc.sync.dma_start(hmt[:], hm[:])
        idt = cpool.tile([128, 128], DTB, tag="id")
        nc.sync.dma_start(idt[:], idm[:])
        wqt = cpool.tile([128, G * OO], DTB, tag="wq")
        nc.sync.dma_start(wqt[:], wqm[:])
        hbt = cpool.tile([128, G * OO], DTB, tag="hb")
        nc.sync.dma_start(hbt[:], hbm[:])

        def body():
            rr = _Drain(nc)
            # contiguous loads: xa_oct[q][p, kk*TOKC + t] = xin[(8q+kk)*128+p, t].
            # One tile + one DMA per octet of k, spread across four DGE queues
            # so the head latency is ~1/4 and iteration n+1 prefetches during n.
            xin_v = xin.rearrange("(k p) t -> p k t", p=128)
            dma_engs = [nc.sync, nc.scalar, nc.gpsimd, nc.sync]
            xa_oct = []
            for q in range(4):
                xo = xapool.tile([128, 8 * TOKC], DTB, tag=f"xa{q}", name=f"xa{q}")
                xo_v = xo.rearrange("p (k t) -> p k t", k=8)
                dma_engs[q].dma_start(xo_v[:], xin_v[:, q * 8 : (q + 1) * 8, :])
                xa_oct.append(xo)

            def xas(k, lo, hi):
                return xa_oct[k // 8][:, (k % 8) * TOKC + lo : (k % 8) * TOKC + hi]

            state = [dict() for _ in range(NST)]

            def p1(st):
                t0 = st * ST
                s = state[st]
                s["tm"] = []
                for c in range(CH):
                    tm = tpool.tile([128, D], DTB, tag=f"tm{c}", name=f"tm{c}")
                    s["tm"].append(tm)
                    for q in range(8):
                        ps = pspool.tile([128, 512], DTF, tag="psA", name="psA", bufs=3)
                        for kk in range(4):
                            k = q * 4 + kk
                            nc.tensor.matmul(
                                ps[:, kk * 128 : (kk + 1) * 128],
                                lhsT=xas(k, t0 + c * 128, t0 + (c + 1) * 128),
                                rhs=hmt[:],
                                start=True,
                                stop=True,
                            )
                        rr(tm[:, q * 512 : (q + 1) * 512], ps[:])
                        yield

            def p2(st):
                s = state[st]
                xb_all = mpool.tile([128, 32 * ST], DTB, tag="xb", name="xb_all")
                s["xb"] = xb_all
                for hg in range(8):  # 4 h per psum tile
                    ps = pspool.tile([128, 512], DTB, tag="psB", name="psB", bufs=3)
                    for j in range(4):
                        h = 4 * hg + j
                        for c in range(CH):
                            sel = s["tm"][c].rearrange("p (i h) -> p h i", h=32)[:, h, :]
                            nc.tensor.transpose(
                                ps[:, j * ST + c * 128 : j * ST + (c + 1) * 128],
                                sel,
                                idt[:],
                            )
                    rr(xb_all[:, hg * 512 : (hg + 1) * 512], ps[:])
                    yield

            def p3(st):
                s = state[st]
                xb_all = s["xb"]
                s["tm2"] = []
                for c in range(CH):
                    tm2 = tpool.tile([128, D], DTB, tag=f"tm2_{c}", name=f"tm2_{c}")
                    s["tm2"].append(tm2)
                    tm2v = tm2.rearrange("p (o h) -> p h o", h=32)
                    for hq in range(8):  # 4 h per psum tile
                        ps = pspool.tile([128, 512], DTF, tag="psA", name="psA", bufs=3)
                        for hh in range(4):
                            h = hq * 4 + hh
                            nc.tensor.matmul(
                                ps[:, hh * 128 : (hh + 1) * 128],
                                lhsT=xb_all[
                                    :, h * ST + c * 128 : h * ST + (c + 1) * 128
                                ],
                                rhs=wqt[:, h * 128 : (h + 1) * 128],
                                start=True,
                                stop=True,
                            )
                        rr(tm2v[:, hq * 4 : (hq + 1) * 4, :], ps[:])
                        yield

            def p4(st):
                s = state[st]
                ypa_all = mpool.tile([128, 32 * ST], DTB, tag="ypa", name="ypa_all")
                s["ypa"] = ypa_all
                for mg in range(8):
                    ps = pspool.tile([128, 512], DTB, tag="psB", name="psB", bufs=3)
                    for j in range(4):
                        m = 4 * mg + j
                        for c in range(CH):
                            nc.tensor.transpose(
                                ps[:, j * ST + c * 128 : j * ST + (c + 1) * 128],
                                s["tm2"][c][:, m * 128 : (m + 1) * 128],
                                idt[:],
                            )
                    rr(ypa_all[:, mg * 512 : (mg + 1) * 512], ps[:])
                    yield

            def p5(st):
                t0 = st * ST
                s = state[st]
                ypa_all = s["ypa"]
                yf_all = ypool.tile([128, 32 * ST], DTB, tag="yf", name="yf_all")
                for mp in range(8):
                    ps = pspool.tile([128, 512], DTF, tag="psC", name="psC", bufs=2)
                    for j in range(4):
                        m = 4 * mp + j
                        nc.tensor.matmul(
                            ps[:, j * ST : (j + 1) * ST],
                            lhsT=hbt[:, m * 128 : (m + 1) * 128],
                            rhs=ypa_all[:, m * ST : (m + 1) * ST],
                            start=True,
                            stop=True,
                        )
                    rr(yf_all[:, mp * 512 : (mp + 1) * 512], ps[:])
                    if mp < 7:
                        yield
                yout_v = yout.rearrange("(m p) t -> p m t", p=128)
                yf_v = yf_all.rearrange("p (m t) -> p m t", m=32)
                nc.gpsimd.dma_start(yout_v[:, :, t0 : t0 + ST], yf_v[:])
                yield

            stages = [p1, p2, p3, p4, p5]
            # software pipeline: slot t runs stage s of supertile t-s. Within
            # a slot, the active stages' per-PSUM-tile units are emitted
            # round-robin so same-tag PSUM allocations are spaced ~5 units
            # apart and drains always finish before the pool wraps.
            for t in range(NST + 4):
                gens = []
                for s in range(4, -1, -1):
                    st = t - s
                    if 0 <= st < NST:
                        gens.append(stages[s](st))
                while gens:
                    alive = []
                    for g in gens:
                        if next(g, StopIteration) is not StopIteration:
                            alive.append(g)
                    gens = alive

        if loop_r == 1:
            body()
        else:
            with tc.For_i(0, loop_r, 1):
                body()


@functools.lru_cache(maxsize=4)
def build_program(loop_r=1):
    nc = bacc.Bacc("TRN2", target_bir_lowering=False, debug=False)
    xin = nc.dram_tensor("xin", [D, TOKC], DTB, kind="ExternalInput").ap()
    hm = nc.dram_tensor("hmat", [128, 128], DTB, kind="ExternalInput").ap()
    idm = nc.dram_tensor("ident", [128, 128], DTB, kind="ExternalInput").ap()
    wqm = nc.dram_tensor("wqm", [128, G * OO], DTB, kind="ExternalInput").ap()
    hbm = nc.dram_tensor("hbm", [128, G * OO], DTB, kind="ExternalInput").ap()
    yout = nc.dram_tensor("yout", [D, TOKC], DTB, kind="ExternalOutput").ap()
    with tile.TileContext(nc) as tc:
        build_body(nc, tc, xin, hm, idm, wqm, hbm, yout, loop_r=loop_r)
    nc.compile()
    return nc


def host_prep(x, weight, alpha, beta):
    """Pure f32 numpy glue + bf16 casts. Returns per-core input maps."""
    H = _hadamard(G)  # [g,h] +-1

    w = np.asarray(weight, dtype=np.float32)
    scale = np.float32(np.mean(np.abs(w))) + np.float32(1e-8)
    wq3 = np.clip(np.round(w / scale), -1.0, 1.0).astype(np.float32)  # [h,o,i]

    # x * alpha, reorder features to i-major (f' = i*32+g), transpose to [D, BT]
    xp = np.asarray(x, dtype=np.float32).reshape(BT, G, IO) * np.asarray(
        alpha, dtype=np.float32
    )[None]
    xt = np.ascontiguousarray(xp.transpose(2, 1, 0)).reshape(D, BT)  # rows i*32+g
    xt = xt.astype(BF16)

    hmat = np.kron(np.eye(4, dtype=np.float32), H).astype(BF16)  # [(i',g),(i'',h)]
    ident = np.eye(128, dtype=np.float32).astype(BF16)
    wq_sb = np.ascontiguousarray(wq3.transpose(2, 0, 1)).reshape(IO, G * OO).astype(BF16)

    # hbm[p=(o''*32+h), m*128 + o'*32 + g] = delta(o'',o') * H[h,g] * bs[4m+o', g]
    bs = np.asarray(beta, dtype=np.float32).T * (scale / np.float32(G))  # [o, g]
    hb = np.zeros((128, G * OO), dtype=np.float32)
    base = np.kron(np.eye(4, dtype=np.float32), H)  # [(o'',h), (o',g)]
    # columns of block m: j = o'*32 + g, scale = bs[4m+o', g]
    for m in range(32):
        colscale = bs[4 * m : 4 * m + 4, :].reshape(128)
        hb[:, m * 128 : (m + 1) * 128] = base * colscale[None, :]
    hb = hb.astype(BF16)

    in_maps = []
    for c in range(NCORES):
        in_maps.append(
            {
                "xin": np.ascontiguousarray(xt[:, c * TOKC : (c + 1) * TOKC]),
                "hmat": hmat,
                "ident": ident,
                "wqm": wq_sb,
                "hbm": hb,
            }
        )
    return in_maps


def host_post(results):
    ydev = np.stack([np.asarray(r["yout"], dtype=np.float32) for r in results])
    # row r = m*128 + o'*32 + g  ->  feature (g, o = 4m+o'); want y[tok, g*128+o]
    y = ydev.reshape(NCORES, G, 4, G, TOKC)  # [c, m, o', g, tok]
    y = y.transpose(0, 4, 3, 1, 2)  # [c, tok, g, m, o']
    y = np.ascontiguousarray(y).reshape(BT, D)
    return y.reshape(B, T, D)


def kernel(x, weight, alpha, beta):
    nc = build_program(loop_r=1)
    in_maps = host_prep(x, weight, alpha, beta)
    res = run_bass_kernel_spmd(nc, in_maps, core_ids=list(range(NCORES)))
    return host_post(res.results)
